# revision 9
# baseline (speedup 1.0000x reference)
"""Trainium2 Bass kernel for nn_ForceMatchingLoss (batch-data-parallel over 8 NeuronCores).

Full inputs (B=256) are sharded along the batch dimension: core i computes
batches [32*i, 32*i+32) and returns [sum_b fd_b, sum_b cons_b]; the host
sums the 8 partials and divides by 256 (the loss is a batch mean, so the
"all-reduce" is a trivial host-side sum of 8 scalars).

v2: full-bf16 matmul pipeline (fp32 inputs are cast to bf16 on-chip).  The
fp32 LOW_HIGH 2-pass matmuls/transposes of v1 dominated the tensor engine
(~166us busy); bf16 runs single-pass with fast weight loads.  PSUM
accumulation stays fp32 and the final reductions stay fp32, keeping the
loss within ~1e-5 of the fp64 reference.
"""

import numpy as np


# ---------------------------------------------------------------------------
# Workaround for this walrus build: CTRL-type instructions (Drain) only accept
# a single sync-wait; TileContext's tail drain aggregates one wait per logical
# processor.  Split the waits across a chain of drains.
# ---------------------------------------------------------------------------
def _install_drain_fix():
    import concourse.tile as tile
    from bass_rust import ScopedClock, SyncInfo

    if getattr(tile.TileContext, "_drain_fix_installed", False):
        return

    def _drain_and_barrier(self, tick_clock, wait_clock):
        drain_inst = self.nc.sync.drain()
        wait_clock.add_sem_waits(
            drain_inst.ins, ScopedClock({None: tick_clock.global_clock})
        )
        si = drain_inst.ins.sync_info
        waits = list(si.on_wait) if si is not None else []
        if len(waits) > 1:
            drain_inst.ins.sync_info = SyncInfo(
                on_wait=waits[:1], on_update=list(si.on_update)
            )
            for i in range(1, len(waits)):
                d = self.nc.sync.drain()
                d.ins.sync_info = SyncInfo(on_wait=waits[i : i + 1], on_update=[])

        self.nc.all_engine_barrier()
        popped = self.nc._tile_sem_poison_stack.pop()
        assert popped is self._sem_poison
        self.nc.clear_and_free_semaphores(list(self.sems.allocated().values()))
        self.nc.all_engine_barrier()

    tile.TileContext._drain_and_barrier = _drain_and_barrier
    tile.TileContext._drain_fix_installed = True


import concourse.bass as bass
import concourse.tile as tile
from concourse import mybir
from concourse.bass import ds, ts
from concourse.masks import make_identity

FP32 = mybir.dt.float32
BF16 = mybir.dt.bfloat16
AX = mybir.AxisListType
ALU = mybir.AluOpType
ACTF = mybir.ActivationFunctionType

B = 32          # batches per core
Q = 16
S = 512
M = 8
D = 128
NCH = 4         # s chunks of 128
GB = 4          # batches per group (packed at 32-row offsets in psum)
NG = B // GB    # 8 groups
SCALE = float(D) ** -0.5
EPS = 1e-8
QD = float(Q * D)


def build_nc():
    nc = bass.Bass("TRN2", target_bir_lowering=False, debug=False)
    q_d = nc.dram_tensor("queries", [B, Q, D], FP32, kind="ExternalInput").ap()
    k_d = nc.dram_tensor("keys", [B, S, D], FP32, kind="ExternalInput").ap()
    v_d = nc.dram_tensor("values", [B, S, D], FP32, kind="ExternalInput").ap()
    kcg_d = nc.dram_tensor("k_cg", [B, M, D], FP32, kind="ExternalInput").ap()
    vcg_d = nc.dram_tensor("v_cg", [B, M, D], FP32, kind="ExternalInput").ap()
    out_d = nc.dram_tensor("out", [1, 2], FP32, kind="ExternalOutput").ap()

    with tile.TileContext(nc) as tc:
        with (
            tc.tile_pool(name="const", bufs=1) as constp,
            tc.tile_pool(name="kv4", bufs=2) as kv4p,
            tc.tile_pool(name="kvb", bufs=2) as kvbp,
            tc.tile_pool(name="kt", bufs=5) as ktp,
            tc.tile_pool(name="sm", bufs=3) as smp,
            tc.tile_pool(name="small", bufs=4) as smallp,
            tc.tile_pool(name="vc", bufs=3) as vcp,
            tc.tile_pool(name="jsb", bufs=4) as jsbp,
            tc.tile_pool(name="okb", bufs=2) as okbp,
            tc.tile_pool(name="scr", bufs=5) as scrp,
            tc.tile_pool(name="psA", bufs=2, space="PSUM") as psA,
            tc.tile_pool(name="psS", bufs=1, space="PSUM") as psS,
            tc.tile_pool(name="psOK", bufs=2, space="PSUM") as psOK,
            tc.tile_pool(name="psJ", bufs=2, space="PSUM") as psJ,
            tc.tile_pool(name="psCG", bufs=1, space="PSUM") as psCG,
        ):
            ident = constp.tile([128, 128], FP32)
            make_identity(nc, ident)
            identB = constp.tile([128, 128], BF16)
            nc.scalar.copy(identB, ident)
            zeroTb = constp.tile([128, 128], BF16)
            nc.scalar.activation(out=zeroTb, in_=ident, func=ACTF.Copy, scale=0.0)
            # accumulator columns: [dot 0:32 | d2 32:64 | c2 64:96 | cons 96:128]
            accum = constp.tile([128, 128], FP32)
            nc.gpsimd.memset(accum, 0.0)
            ones1 = constp.tile([128, 1], FP32)
            nc.vector.memset(ones1, 1.0)
            ccg_all = constp.tile([8, 32], FP32)

            # ---------- prologue: q transposed, cg tensors ----------
            q_sb = constp.tile([128, 4, 128], FP32)
            nc.sync.dma_start(
                out=q_sb,
                in_=q_d.rearrange("(t b2) q d -> (b2 q) t d", t=4),
            )
            q_sbb = constp.tile([128, 4, 128], BF16)
            nc.scalar.copy(q_sbb, q_sb)
            qT = constp.tile([128, 4, 128], BF16)  # [d, t, b2*16+q], scaled
            qtps = psA.tile([128, 1024], BF16, tag="tp")
            for t in range(4):
                nc.tensor.transpose(qtps[:, ds(128 * t, 128)], q_sbb[:, t, :], identB)
            # fold the softmax temperature into qT (used by dense + cg scores)
            nc.scalar.activation(
                out=qT[:],
                in_=qtps[:, 0:512].rearrange("p (t x) -> p t x", t=4),
                func=ACTF.Copy,
                scale=SCALE,
            )

            kcg_sb = constp.tile([128, 2, 128], FP32)  # [(b2 m), t, d]
            nc.sync.dma_start(
                out=kcg_sb,
                in_=kcg_d.rearrange("(t b2) m d -> (b2 m) t d", t=2),
            )
            kcg_sbb = constp.tile([128, 2, 128], BF16)
            nc.scalar.copy(kcg_sbb, kcg_sb)
            kcgT = constp.tile([128, 2, 128], BF16)  # [d, t, b2*8+m]
            kcgtps = psA.tile([128, 1024], BF16, tag="tp")
            for t in range(2):
                nc.tensor.transpose(
                    kcgtps[:, ds(128 * t, 128)], kcg_sbb[:, t, :], identB
                )
            nc.scalar.copy(
                kcgT[:], kcgtps[:, 0:256].rearrange("p (t x) -> p t x", t=2)
            )

            # [m, b, {k|v}]: fp32 HWDGE load, then one SWDGE SBUF->SBUF
            # cast to bf16 (large contiguous descriptors, cheap on Q7)
            cgkv4 = constp.tile([8, 32, 256], FP32)
            nc.sync.dma_start(
                out=cgkv4[:, :, 0:128], in_=kcg_d.rearrange("b m d -> m b d")
            )
            nc.sync.dma_start(
                out=cgkv4[:, :, 128:256], in_=vcg_d.rearrange("b m d -> m b d")
            )
            cgkv2 = constp.tile([8, 32, 256], BF16)
            nc.gpsimd.dma_start(out=cgkv2, in_=cgkv4)

            # ---------- main loop over groups of 4 batches ----------
            for g in range(NG):
                bs = [g * GB + j for j in range(GB)]

                # fp32 staging loads (per-batch 256KB DMAs; the AP balancer
                # cannot handle the 4-dim whole-group transfer)
                kv4 = kv4p.tile([128, GB, NCH, 2, 128], FP32, tag="kv4")
                for j, b in enumerate(bs):
                    nc.sync.dma_start(
                        out=kv4[:, j, :, 0, :],
                        in_=k_d[b].rearrange("(p c) d -> p c d", c=NCH),
                    )
                    nc.sync.dma_start(
                        out=kv4[:, j, :, 1, :],
                        in_=v_d[b].rearrange("(p c) d -> p c d", c=NCH),
                    )
                # cast to bf16, split across engines
                kvb = kvbp.tile([128, GB, NCH, 2, 128], BF16, tag="kvb")
                nc.scalar.copy(kvb[:, 0], kv4[:, 0])
                nc.vector.tensor_copy(kvb[:, 1], kv4[:, 1])
                nc.gpsimd.tensor_copy(kvb[:, 2], kv4[:, 2])
                nc.gpsimd.tensor_copy(kvb[:, 3], kv4[:, 3])

                # kT per batch via PE transpose (bf16 single-pass)
                kts = []
                for j in range(GB):
                    ktps = psA.tile([128, 1024], BF16, tag="tp")
                    for c in range(NCH):
                        nc.tensor.transpose(
                            ktps[:, ds(128 * c, 128)], kvb[:, j, c, 0, :], identB
                        )
                    kt = ktp.tile([128, NCH, 128], BF16, tag="kt")
                    view = ktps[:, 0:512].rearrange("p (c x) -> p c x", c=NCH)
                    if j % 2 == 0:
                        nc.scalar.copy(kt[:], view)
                    else:
                        nc.vector.tensor_copy(kt[:], view)
                    kts.append(kt)

                # scores: 4 batches packed at 32-aligned row offsets
                scps = psS.tile([128, 512], FP32, tag="scps")
                nc.tensor.matmul(
                    scps,
                    lhsT=zeroTb,
                    rhs=kts[0][:],
                    start=True,
                    stop=False,
                    skip_group_check=True,
                )
                for j, b in enumerate(bs):
                    t, i = b // 8, b % 8
                    nc.tensor.matmul(
                        scps[ds(32 * j, 16), :],
                        lhsT=qT[:, t, ds(16 * i, 16)],
                        rhs=kts[j][:],
                        start=False,
                        stop=True,
                        tile_position=(0, 32 * j),
                        skip_group_check=True,
                    )

                # softmax (no max subtraction: |scores| <= ~7)
                ptil = smp.tile([128, 512], BF16, tag="ptil")
                z = smallp.tile([128, 1], FP32, tag="z")
                nc.scalar.activation(out=ptil, in_=scps, func=ACTF.Exp, accum_out=z)
                zr = smallp.tile([128, 1], FP32, tag="zr")
                nc.vector.reciprocal(zr, z)
                nc.vector.tensor_scalar_mul(ptil, ptil, zr)

                # pT via PE transpose
                ptps = psA.tile([128, 1024], BF16, tag="tp")
                for c in range(NCH):
                    nc.tensor.transpose(
                        ptps[:, ds(128 * c, 128)], ptil[:, ds(128 * c, 128)], identB
                    )
                pT = smp.tile([128, NCH, 128], BF16, tag="pT")
                nc.scalar.copy(
                    pT[:], ptps[:, 0:512].rearrange("p (c x) -> p c x", c=NCH)
                )

                # c = sum_q p (valid q columns only), scaled by SCALE
                c_t = smallp.tile([128, NCH, GB], FP32, tag="c_t")
                nc.vector.tensor_reduce(
                    out=c_t,
                    in_=pT.rearrange("p c (j w) -> p c j w", j=GB)[:, :, :, 0:16],
                    axis=AX.X,
                    op=ALU.add,
                )
                nc.vector.tensor_scalar_mul(c_t, c_t, SCALE)

                # out/kbar: 4 batches col-packed at 32-row offsets in one bank;
                # cols [kbar 0:128 | out 128:256]
                okps = psOK.tile([128, 512], FP32, tag="okps")
                nc.tensor.matmul(
                    okps[:, 0:256],
                    lhsT=zeroTb,
                    rhs=kvb[:, 0, 0].rearrange("p a x -> p (a x)"),
                    start=True,
                    stop=False,
                    skip_group_check=True,
                )
                for c in range(NCH):
                    for j in range(GB):
                        nc.tensor.matmul(
                            okps[ds(32 * j, 16), 0:256],
                            lhsT=pT[:, c, ds(32 * j, 16)],
                            rhs=kvb[:, j, c].rearrange("p a x -> p (a x)"),
                            start=False,
                            stop=(c == NCH - 1),
                            tile_position=(0, 32 * j),
                            skip_group_check=True,
                        )
                # okb rows 32j: [-s*kbar 0:128 | out 128:256] (bf16)
                okb = okbp.tile([128, 256], BF16, tag="okb")
                nc.scalar.activation(
                    out=okb[:, 0:128], in_=okps[:, 0:128], func=ACTF.Copy,
                    scale=-SCALE,
                )
                nc.scalar.copy(okb[:, 128:256], okps[:, 128:256])
                # move each batch's 16 rows down to partitions 0:16 so the
                # consistency sub and jac2 see base-partition-0 operands
                okb2 = okbp.tile([16, 1024], BF16, tag="okb2")
                for j in range(GB):
                    nc.sync.dma_start(
                        out=okb2[:, ds(256 * j, 256)], in_=okb[ds(32 * j, 16), :]
                    )

                # ---- coarse-grained chain ----
                cgps = psCG.tile([128, 512], FP32, tag="cgps")
                nc.tensor.matmul(
                    cgps[:, 0:8],
                    lhsT=zeroTb,
                    rhs=kcgT[:, 0, 0:8],
                    start=True,
                    stop=False,
                    skip_group_check=True,
                )
                for j, b in enumerate(bs):
                    t2, i2 = b // 16, b % 16
                    nc.tensor.matmul(
                        cgps[ds(32 * j, 16), 0:8],
                        lhsT=qT[:, b // 8, ds(16 * (b % 8), 16)],
                        rhs=kcgT[:, t2, ds(8 * i2, 8)],
                        start=False,
                        stop=True,
                        tile_position=(0, 32 * j),
                        skip_group_check=True,
                    )
                pcg = smallp.tile([128, 8], BF16, tag="pcg")
                zcg = smallp.tile([128, 1], FP32, tag="zcg")
                nc.scalar.activation(
                    out=pcg, in_=cgps[:, 0:8], func=ACTF.Exp, accum_out=zcg
                )
                zcgr = smallp.tile([128, 1], FP32, tag="zcgr")
                nc.vector.reciprocal(zcgr, zcg)
                nc.vector.tensor_scalar_mul(pcg, pcg, zcgr)
                pcgtps = psA.tile([128, 1024], BF16, tag="tp")
                nc.tensor.transpose(pcgtps[0:8, 0:128], pcg, identB)
                pcgT = smallp.tile([8, 128], BF16, tag="pcgT")
                nc.scalar.copy(pcgT[:], pcgtps[0:8, 0:128])
                nc.vector.tensor_reduce(
                    out=ccg_all[:, ds(GB * g, GB)],
                    in_=pcgT.rearrange("m (j w) -> m j w", j=GB)[:, :, 0:16],
                    axis=AX.X,
                    op=ALU.add,
                )
                # cg out/kbar: psum cols 256:512 = [kbar_cg | out_cg], rows 32j
                nc.tensor.matmul(
                    cgps[:, 256:512],
                    lhsT=zeroTb[0:8, :],
                    rhs=cgkv2[:, bs[0], :],
                    start=True,
                    stop=False,
                    skip_group_check=True,
                )
                for j, b in enumerate(bs):
                    nc.tensor.matmul(
                        cgps[ds(32 * j, 16), 256:512],
                        lhsT=pcgT[:, ds(32 * j, 16)],
                        rhs=cgkv2[:, b, :],
                        start=False,
                        stop=True,
                        tile_position=(0, 32 * j),
                        skip_group_check=True,
                    )
                # okcg rows 32j: [-s*kbar_cg 0:128 | out_cg 128:256] (bf16)
                okcg = okbp.tile([128, 256], BF16, tag="okcg")
                nc.scalar.activation(
                    out=okcg[:, 0:128], in_=cgps[:, 256:384], func=ACTF.Copy,
                    scale=-SCALE,
                )
                nc.scalar.copy(okcg[:, 128:256], cgps[:, 384:512])
                okcg2 = okbp.tile([16, 1024], BF16, tag="okcg2")
                for j in range(GB):
                    nc.sync.dma_start(
                        out=okcg2[:, ds(256 * j, 256)], in_=okcg[ds(32 * j, 16), :]
                    )

                # consistency per batch: out (okb2) vs out_cg (okcg2), rows 0:16
                for j, b in enumerate(bs):
                    dif = scrp.tile([16, 128], BF16, tag="dif")
                    nc.vector.tensor_sub(
                        dif,
                        okb2[:, ds(256 * j + 128, 128)],
                        okcg2[:, ds(256 * j + 128, 128)],
                    )
                    scc = scrp.tile([16, 128], BF16, tag="scc")
                    nc.vector.scalar_tensor_tensor(
                        out=scc,
                        in0=dif,
                        scalar=1.0,
                        in1=dif,
                        op0=ALU.mult,
                        op1=ALU.mult,
                        accum_out=accum[0:16, ds(96 + b, 1)],
                    )

                # ---- per-batch jacobians: jp cols [dense 0:128 | cg 128:256] ----
                for j, b in enumerate(bs):
                    vc = vcp.tile([128, NCH, 128], BF16, tag="vc")
                    nc.gpsimd.tensor_tensor(
                        out=vc[:],
                        in0=kvb[:, j, :, 1, :],
                        in1=c_t[:, :, ds(j, 1)].broadcast_to([128, NCH, 128]),
                        op=ALU.mult,
                    )
                    vccg = smallp.tile([8, 128], BF16, tag="vccg")
                    nc.vector.tensor_scalar(
                        out=vccg[:],
                        in0=cgkv2[:, b, 128:256],
                        scalar1=ccg_all[:, ds(b, 1)],
                        scalar2=SCALE,
                        op0=ALU.mult,
                        op1=ALU.mult,
                    )

                    jp = psJ.tile([128, 512], FP32, tag="jp")
                    for c in range(NCH):
                        nc.tensor.matmul(
                            jp[:, 0:128],
                            lhsT=vc[:, c, :],
                            rhs=kvb[:, j, c, 0, :],
                            start=(c == 0),
                            stop=False,
                            skip_group_check=True,
                        )
                    nc.tensor.matmul(
                        jp[:, 0:128],
                        lhsT=okb2[:, ds(256 * j + 128, 128)],
                        rhs=okb2[:, ds(256 * j, 128)],
                        start=False,
                        stop=True,
                        skip_group_check=True,
                    )
                    nc.tensor.matmul(
                        jp[:, 128:256],
                        lhsT=vccg,
                        rhs=cgkv2[:, b, 0:128],
                        start=True,
                        stop=False,
                        skip_group_check=True,
                    )
                    nc.tensor.matmul(
                        jp[:, 128:256],
                        lhsT=okcg2[:, ds(256 * j + 128, 128)],
                        rhs=okcg2[:, ds(256 * j, 128)],
                        start=False,
                        stop=True,
                        skip_group_check=True,
                    )

                    jsb = jsbp.tile([128, 256], BF16, tag="jsb")
                    nc.scalar.copy(jsb[:], jp[:, 0:256])

                    s1 = scrp.tile([128, 128], BF16, tag="s1")
                    nc.vector.scalar_tensor_tensor(
                        out=s1, in0=jsb[:, 0:128], scalar=1.0,
                        in1=jsb[:, 128:256],
                        op0=ALU.mult, op1=ALU.mult,
                        accum_out=accum[:, ds(b, 1)],
                    )
                    s2 = scrp.tile([128, 128], BF16, tag="s2")
                    nc.scalar.activation(
                        out=s2, in_=jsb[:, 128:256], func=ACTF.Square,
                        accum_out=accum[:, ds(64 + b, 1)],
                    )
                    s3 = scrp.tile([128, 128], BF16, tag="s3")
                    nc.scalar.activation(
                        out=s3, in_=jsb[:, 0:128], func=ACTF.Square,
                        accum_out=accum[:, ds(32 + b, 1)],
                    )

            # ---------- final reduction ----------
            # partition reduction via ones-vector matmul (gpsimd C-reduce is
            # pathologically slow on hardware)
            rps = psJ.tile([1, 128], FP32, tag="jp")
            nc.tensor.matmul(
                rps, lhsT=ones1, rhs=accum, start=True, stop=True,
                skip_group_check=True,
            )
            row = constp.tile([1, 128], FP32)
            nc.scalar.copy(row, rps)
            f1 = constp.tile([1, 32], FP32)
            nc.vector.tensor_tensor(
                out=f1, in0=row[:, 32:64], in1=row[:, 64:96], op=ALU.mult
            )
            nc.scalar.activation(out=f1, in_=f1, func=ACTF.Sqrt)
            nc.vector.tensor_scalar_add(f1, f1, EPS)
            f2 = constp.tile([1, 32], FP32)
            nc.vector.reciprocal(f2, f1)
            nc.vector.tensor_tensor(
                out=f2, in0=row[:, 0:32], in1=f2, op=ALU.mult
            )
            csum = constp.tile([1, 1], FP32)
            nc.vector.tensor_reduce(out=csum, in_=f2, axis=AX.X, op=ALU.add)
            msum = constp.tile([1, 1], FP32)
            nc.vector.tensor_reduce(
                out=msum, in_=row[:, 96:128], axis=AX.X, op=ALU.add
            )
            part = constp.tile([1, 2], FP32)
            nc.vector.tensor_scalar(
                out=part[:, 0:1], in0=csum, scalar1=-1.0, scalar2=float(B),
                op0=ALU.mult, op1=ALU.add,
            )
            nc.vector.tensor_scalar_mul(part[:, 1:2], msum, 1.0 / QD)
            nc.sync.dma_start(out=out_d, in_=part)

    return nc


_NC_CACHE = {}


def _get_nc():
    if "nc" not in _NC_CACHE:
        _install_drain_fix()
        nc = build_nc()
        _split_waits(nc)
        _NC_CACHE["nc"] = nc
    return _NC_CACHE["nc"]


def _split_waits(nc):
    """This walrus accepts only one sync-wait per instruction; move extras
    onto same-engine NoOps inserted just before."""
    from concourse import mybir
    from bass_rust import SyncInfo

    for f in nc.m.functions:
        for blk in f.blocks:
            insts = list(blk.instructions)
            out = []
            for inst in insts:
                si = inst.sync_info
                waits = list(si.on_wait) if si is not None else []
                if len(waits) > 1:
                    for wi, w in enumerate(waits[:-1]):
                        nop = mybir.InstNoOp(name=f"{inst.name}-wsplit{wi}")
                        nop.engine = inst.engine
                        nop.sync_info = SyncInfo(on_wait=[w], on_update=[])
                        out.append(nop)
                    inst.sync_info = SyncInfo(
                        on_wait=[waits[-1]], on_update=list(si.on_update)
                    )
                out.append(inst)
            blk.instructions = out


N_CORES = 8


def kernel(queries, keys, values, k_cg, v_cg):
    from concourse.bass_utils import run_bass_kernel_spmd

    queries = np.ascontiguousarray(np.asarray(queries, dtype=np.float32))
    keys = np.ascontiguousarray(np.asarray(keys, dtype=np.float32))
    values = np.ascontiguousarray(np.asarray(values, dtype=np.float32))
    k_cg = np.ascontiguousarray(np.asarray(k_cg, dtype=np.float32))
    v_cg = np.ascontiguousarray(np.asarray(v_cg, dtype=np.float32))

    nb = queries.shape[0]
    sh = nb // N_CORES
    in_maps = [
        {
            "queries": queries[i * sh : (i + 1) * sh],
            "keys": keys[i * sh : (i + 1) * sh],
            "values": values[i * sh : (i + 1) * sh],
            "k_cg": k_cg[i * sh : (i + 1) * sh],
            "v_cg": v_cg[i * sh : (i + 1) * sh],
        }
        for i in range(N_CORES)
    ]
    nc = _get_nc()
    res = run_bass_kernel_spmd(nc, in_maps, core_ids=list(range(N_CORES)))
    total = 0.0
    for i in range(N_CORES):
        part = res.results[i]["out"]
        total += float(part[0, 0]) + float(part[0, 1])
    return np.float32(total / nb)


# revision 14
# speedup vs baseline: 1.1770x; 1.1770x over previous
"""Trainium2 Bass kernel for nn_ForceMatchingLoss (batch-data-parallel over 8 NeuronCores).

Full inputs (B=256) are sharded along the batch dimension: core i computes
batches [32*i, 32*i+32) and returns [sum_b fd_b, sum_b cons_b]; the host
sums the 8 partials and divides by 256 (the loss is a batch mean, so the
"all-reduce" is a trivial host-side sum of 8 scalars).

v2: full-bf16 matmul pipeline (fp32 inputs are cast to bf16 on-chip).  The
fp32 LOW_HIGH 2-pass matmuls/transposes of v1 dominated the tensor engine
(~166us busy); bf16 runs single-pass with fast weight loads.  PSUM
accumulation stays fp32 and the final reductions stay fp32, keeping the
loss within ~1e-5 of the fp64 reference.
"""

import numpy as np


# ---------------------------------------------------------------------------
# Workaround for this walrus build: CTRL-type instructions (Drain) only accept
# a single sync-wait; TileContext's tail drain aggregates one wait per logical
# processor.  Split the waits across a chain of drains.
# ---------------------------------------------------------------------------
def _install_drain_fix():
    import concourse.tile as tile
    from bass_rust import ScopedClock, SyncInfo

    if getattr(tile.TileContext, "_drain_fix_installed", False):
        return

    def _drain_and_barrier(self, tick_clock, wait_clock):
        drain_inst = self.nc.sync.drain()
        wait_clock.add_sem_waits(
            drain_inst.ins, ScopedClock({None: tick_clock.global_clock})
        )
        si = drain_inst.ins.sync_info
        waits = list(si.on_wait) if si is not None else []
        if len(waits) > 1:
            drain_inst.ins.sync_info = SyncInfo(
                on_wait=waits[:1], on_update=list(si.on_update)
            )
            for i in range(1, len(waits)):
                d = self.nc.sync.drain()
                d.ins.sync_info = SyncInfo(on_wait=waits[i : i + 1], on_update=[])

        self.nc.all_engine_barrier()
        popped = self.nc._tile_sem_poison_stack.pop()
        assert popped is self._sem_poison
        self.nc.clear_and_free_semaphores(list(self.sems.allocated().values()))
        self.nc.all_engine_barrier()

    tile.TileContext._drain_and_barrier = _drain_and_barrier
    tile.TileContext._drain_fix_installed = True


import concourse.bass as bass
import concourse.tile as tile
from concourse import mybir
from concourse.bass import ds, ts
from concourse.masks import make_identity

FP32 = mybir.dt.float32
BF16 = mybir.dt.bfloat16
AX = mybir.AxisListType
ALU = mybir.AluOpType
ACTF = mybir.ActivationFunctionType

B = 32          # batches per core
Q = 16
S = 512
M = 8
D = 128
NCH = 4         # s chunks of 128
GB = 4          # batches per group (packed at 32-row offsets in psum)
NG = B // GB    # 8 groups
SCALE = float(D) ** -0.5
EPS = 1e-8
QD = float(Q * D)


def build_nc():
    nc = bass.Bass("TRN2", target_bir_lowering=False, debug=False)
    q_d = nc.dram_tensor("queries", [B, Q, D], FP32, kind="ExternalInput").ap()
    k_d = nc.dram_tensor("keys", [B, S, D], FP32, kind="ExternalInput").ap()
    v_d = nc.dram_tensor("values", [B, S, D], FP32, kind="ExternalInput").ap()
    kcg_d = nc.dram_tensor("k_cg", [B, M, D], FP32, kind="ExternalInput").ap()
    vcg_d = nc.dram_tensor("v_cg", [B, M, D], FP32, kind="ExternalInput").ap()
    out_d = nc.dram_tensor("out", [1, 2], FP32, kind="ExternalOutput").ap()

    with tile.TileContext(nc) as tc:
        with (
            tc.tile_pool(name="const", bufs=1) as constp,
            tc.tile_pool(name="kv4", bufs=2) as kv4p,
            tc.tile_pool(name="kvb", bufs=2) as kvbp,
            tc.tile_pool(name="kt", bufs=5) as ktp,
            tc.tile_pool(name="sm", bufs=3) as smp,
            tc.tile_pool(name="small", bufs=4) as smallp,
            tc.tile_pool(name="vc", bufs=3) as vcp,
            tc.tile_pool(name="jsb", bufs=4) as jsbp,
            tc.tile_pool(name="okb", bufs=2) as okbp,
            tc.tile_pool(name="scr", bufs=5) as scrp,
            tc.tile_pool(name="psA", bufs=2, space="PSUM") as psA,
            tc.tile_pool(name="psS", bufs=1, space="PSUM") as psS,
            tc.tile_pool(name="psOK", bufs=2, space="PSUM") as psOK,
            tc.tile_pool(name="psJ", bufs=2, space="PSUM") as psJ,
            tc.tile_pool(name="psCG", bufs=1, space="PSUM") as psCG,
        ):
            ident = constp.tile([128, 128], FP32)
            make_identity(nc, ident)
            identB = constp.tile([128, 128], BF16)
            nc.scalar.copy(identB, ident)
            zeroTb = constp.tile([128, 128], BF16)
            nc.scalar.activation(out=zeroTb, in_=ident, func=ACTF.Copy, scale=0.0)
            # accumulator columns: [dot 0:32 | d2 32:64 | c2 64:96 | cons 96:128]
            accum = constp.tile([128, 128], FP32)
            nc.gpsimd.memset(accum, 0.0)
            ones1 = constp.tile([128, 1], FP32)
            nc.vector.memset(ones1, 1.0)
            ccg_all = constp.tile([8, 32], FP32)

            # ---------- prologue: q transposed, cg tensors ----------
            q_sb = constp.tile([128, 4, 128], FP32)
            nc.sync.dma_start(
                out=q_sb,
                in_=q_d.rearrange("(t b2) q d -> (b2 q) t d", t=4),
            )
            q_sbb = constp.tile([128, 4, 128], BF16)
            nc.scalar.copy(q_sbb, q_sb)
            qT = constp.tile([128, 4, 128], BF16)  # [d, t, b2*16+q], scaled
            qtps = psA.tile([128, 1024], BF16, tag="tp")
            for t in range(4):
                nc.tensor.transpose(qtps[:, ds(128 * t, 128)], q_sbb[:, t, :], identB)
            # fold the softmax temperature into qT (used by dense + cg scores)
            nc.scalar.activation(
                out=qT[:],
                in_=qtps[:, 0:512].rearrange("p (t x) -> p t x", t=4),
                func=ACTF.Copy,
                scale=SCALE,
            )

            kcg_sb = constp.tile([128, 2, 128], FP32)  # [(b2 m), t, d]
            nc.sync.dma_start(
                out=kcg_sb,
                in_=kcg_d.rearrange("(t b2) m d -> (b2 m) t d", t=2),
            )
            kcg_sbb = constp.tile([128, 2, 128], BF16)
            nc.scalar.copy(kcg_sbb, kcg_sb)
            kcgT = constp.tile([128, 2, 128], BF16)  # [d, t, b2*8+m]
            kcgtps = psA.tile([128, 1024], BF16, tag="tp")
            for t in range(2):
                nc.tensor.transpose(
                    kcgtps[:, ds(128 * t, 128)], kcg_sbb[:, t, :], identB
                )
            nc.scalar.copy(
                kcgT[:], kcgtps[:, 0:256].rearrange("p (t x) -> p t x", t=2)
            )

            # [m, b, {k|v}]: fp32 HWDGE load, then one SWDGE SBUF->SBUF
            # cast to bf16 (large contiguous descriptors, cheap on Q7)
            cgkv4 = constp.tile([8, 32, 256], FP32)
            nc.sync.dma_start(
                out=cgkv4[:, :, 0:128], in_=kcg_d.rearrange("b m d -> m b d")
            )
            nc.sync.dma_start(
                out=cgkv4[:, :, 128:256], in_=vcg_d.rearrange("b m d -> m b d")
            )
            cgkv2 = constp.tile([8, 32, 256], BF16)
            nc.gpsimd.dma_start(out=cgkv2, in_=cgkv4)

            # ---------- main loop over groups of 4 batches ----------
            for g in range(NG):
                bs = [g * GB + j for j in range(GB)]

                # fp32 staging loads (per-batch 256KB DMAs; the AP balancer
                # cannot handle the 4-dim whole-group transfer)
                kv4 = kv4p.tile([128, GB, NCH, 2, 128], FP32, tag="kv4")
                for j, b in enumerate(bs):
                    nc.sync.dma_start(
                        out=kv4[:, j, :, 0, :],
                        in_=k_d[b].rearrange("(p c) d -> p c d", c=NCH),
                    )
                    nc.sync.dma_start(
                        out=kv4[:, j, :, 1, :],
                        in_=v_d[b].rearrange("(p c) d -> p c d", c=NCH),
                    )
                # cast to bf16 (vector CAST is by far the cheapest; gpsimd
                # CAST measured 3.7us -- never use it)
                kvb = kvbp.tile([128, GB, NCH, 2, 128], BF16, tag="kvb")
                nc.scalar.copy(kvb[:, 0], kv4[:, 0])
                nc.vector.tensor_copy(kvb[:, 1], kv4[:, 1])
                nc.vector.tensor_copy(kvb[:, 2], kv4[:, 2])
                nc.vector.tensor_copy(kvb[:, 3], kv4[:, 3])

                # kT per batch via PE transpose (bf16 single-pass)
                kts = []
                for j in range(GB):
                    ktps = psA.tile([128, 1024], BF16, tag="tp")
                    for c in range(NCH):
                        nc.tensor.transpose(
                            ktps[:, ds(128 * c, 128)], kvb[:, j, c, 0, :], identB
                        )
                    kt = ktp.tile([128, NCH, 128], BF16, tag="kt")
                    view = ktps[:, 0:512].rearrange("p (c x) -> p c x", c=NCH)
                    if j % 2 == 0:
                        nc.scalar.copy(kt[:], view)
                    else:
                        nc.vector.tensor_copy(kt[:], view)
                    kts.append(kt)

                # scores: 4 batches packed at 32-aligned row offsets
                scps = psS.tile([128, 512], FP32, tag="scps")
                nc.tensor.matmul(
                    scps,
                    lhsT=zeroTb,
                    rhs=kts[0][:],
                    start=True,
                    stop=False,
                    skip_group_check=True,
                )
                for j, b in enumerate(bs):
                    t, i = b // 8, b % 8
                    nc.tensor.matmul(
                        scps[ds(32 * j, 16), :],
                        lhsT=qT[:, t, ds(16 * i, 16)],
                        rhs=kts[j][:],
                        start=False,
                        stop=True,
                        tile_position=(0, 32 * j),
                        skip_group_check=True,
                    )

                # softmax (no max subtraction: |scores| <= ~7)
                ptil = smp.tile([128, 512], BF16, tag="ptil")
                z = smallp.tile([128, 1], FP32, tag="z")
                nc.scalar.activation(out=ptil, in_=scps, func=ACTF.Exp, accum_out=z)
                zr = smallp.tile([128, 1], FP32, tag="zr")
                nc.vector.reciprocal(zr, z)
                nc.vector.tensor_scalar_mul(ptil, ptil, zr)

                # pT via PE transpose
                ptps = psA.tile([128, 1024], BF16, tag="tp")
                for c in range(NCH):
                    nc.tensor.transpose(
                        ptps[:, ds(128 * c, 128)], ptil[:, ds(128 * c, 128)], identB
                    )
                pT = smp.tile([128, NCH, 128], BF16, tag="pT")
                nc.scalar.copy(
                    pT[:], ptps[:, 0:512].rearrange("p (c x) -> p c x", c=NCH)
                )

                # c = sum_q p (valid q columns only).  NOTE: the jacobians are
                # computed WITHOUT the overall SCALE factor -- cosine
                # similarity is invariant to a uniform scale on each jacobian,
                # so SCALE is dropped from c/vc/ccg/vccg and the jac2 kbar
                # copies use -1.0 below.
                c_t = smallp.tile([128, NCH, GB], FP32, tag="c_t")
                nc.vector.tensor_reduce(
                    out=c_t,
                    in_=pT.rearrange("p c (j w) -> p c j w", j=GB)[:, :, :, 0:16],
                    axis=AX.X,
                    op=ALU.add,
                )

                # out/kbar: 4 batches col-packed at 32-row offsets in one bank;
                # cols [kbar 0:128 | out 128:256]
                okps = psOK.tile([128, 512], FP32, tag="okps")
                nc.tensor.matmul(
                    okps[:, 0:256],
                    lhsT=zeroTb,
                    rhs=kvb[:, 0, 0].rearrange("p a x -> p (a x)"),
                    start=True,
                    stop=False,
                    skip_group_check=True,
                )
                for c in range(NCH):
                    for j in range(GB):
                        nc.tensor.matmul(
                            okps[ds(32 * j, 16), 0:256],
                            lhsT=pT[:, c, ds(32 * j, 16)],
                            rhs=kvb[:, j, c].rearrange("p a x -> p (a x)"),
                            start=False,
                            stop=(c == NCH - 1),
                            tile_position=(0, 32 * j),
                            skip_group_check=True,
                        )
                # okb rows 32j: [-s*kbar 0:128 | out 128:256] (bf16)
                okb = okbp.tile([128, 256], BF16, tag="okb")
                nc.scalar.activation(
                    out=okb[:, 0:128], in_=okps[:, 0:128], func=ACTF.Copy,
                    scale=-1.0,
                )
                nc.scalar.copy(okb[:, 128:256], okps[:, 128:256])
                # move each batch's 16 rows down to partitions 0:16 so the
                # consistency sub and jac2 see base-partition-0 operands
                okb2 = okbp.tile([16, 1024], BF16, tag="okb2")
                for j in range(GB):
                    nc.sync.dma_start(
                        out=okb2[:, ds(256 * j, 256)], in_=okb[ds(32 * j, 16), :]
                    )

                # ---- coarse-grained chain ----
                cgps = psCG.tile([128, 512], FP32, tag="cgps")
                nc.tensor.matmul(
                    cgps[:, 0:8],
                    lhsT=zeroTb,
                    rhs=kcgT[:, 0, 0:8],
                    start=True,
                    stop=False,
                    skip_group_check=True,
                )
                for j, b in enumerate(bs):
                    t2, i2 = b // 16, b % 16
                    nc.tensor.matmul(
                        cgps[ds(32 * j, 16), 0:8],
                        lhsT=qT[:, b // 8, ds(16 * (b % 8), 16)],
                        rhs=kcgT[:, t2, ds(8 * i2, 8)],
                        start=False,
                        stop=True,
                        tile_position=(0, 32 * j),
                        skip_group_check=True,
                    )
                pcgf = smallp.tile([128, 8], FP32, tag="pcgf")
                zcg = smallp.tile([128, 1], FP32, tag="zcg")
                nc.scalar.activation(
                    out=pcgf, in_=cgps[:, 0:8], func=ACTF.Exp, accum_out=zcg
                )
                zcgr = smallp.tile([128, 1], FP32, tag="zcgr")
                nc.vector.reciprocal(zcgr, zcg)
                pcg = smallp.tile([128, 8], BF16, tag="pcg")
                nc.vector.tensor_scalar_mul(pcg, pcgf, zcgr)
                pcgtps = psA.tile([128, 1024], BF16, tag="tp")
                nc.tensor.transpose(pcgtps[0:8, 0:128], pcg, identB)
                pcgT = smallp.tile([8, 128], BF16, tag="pcgT")
                nc.scalar.copy(pcgT[:], pcgtps[0:8, 0:128])
                nc.vector.tensor_reduce(
                    out=ccg_all[:, ds(GB * g, GB)],
                    in_=pcgT.rearrange("m (j w) -> m j w", j=GB)[:, :, 0:16],
                    axis=AX.X,
                    op=ALU.add,
                )
                # cg out/kbar: psum cols 256:512 = [kbar_cg | out_cg], rows 32j
                nc.tensor.matmul(
                    cgps[:, 256:512],
                    lhsT=zeroTb[0:8, :],
                    rhs=cgkv2[:, bs[0], :],
                    start=True,
                    stop=False,
                    skip_group_check=True,
                )
                for j, b in enumerate(bs):
                    nc.tensor.matmul(
                        cgps[ds(32 * j, 16), 256:512],
                        lhsT=pcgT[:, ds(32 * j, 16)],
                        rhs=cgkv2[:, b, :],
                        start=False,
                        stop=True,
                        tile_position=(0, 32 * j),
                        skip_group_check=True,
                    )
                # okcg rows 32j: [-s*kbar_cg 0:128 | out_cg 128:256] (bf16)
                okcg = okbp.tile([128, 256], BF16, tag="okcg")
                nc.scalar.activation(
                    out=okcg[:, 0:128], in_=cgps[:, 256:384], func=ACTF.Copy,
                    scale=-1.0,
                )
                nc.scalar.copy(okcg[:, 128:256], cgps[:, 384:512])
                okcg2 = okbp.tile([16, 1024], BF16, tag="okcg2")
                for j in range(GB):
                    nc.sync.dma_start(
                        out=okcg2[:, ds(256 * j, 256)], in_=okcg[ds(32 * j, 16), :]
                    )

                # consistency per batch: out (okb2) vs out_cg (okcg2), rows 0:16
                for j, b in enumerate(bs):
                    dif = scrp.tile([16, 128], BF16, tag="dif")
                    nc.vector.tensor_sub(
                        dif,
                        okb2[:, ds(256 * j + 128, 128)],
                        okcg2[:, ds(256 * j + 128, 128)],
                    )
                    scc = scrp.tile([16, 128], BF16, tag="scc")
                    nc.vector.scalar_tensor_tensor(
                        out=scc,
                        in0=dif,
                        scalar=1.0,
                        in1=dif,
                        op0=ALU.mult,
                        op1=ALU.mult,
                        accum_out=accum[0:16, ds(96 + b, 1)],
                    )

                # ---- per-batch jacobians: jp cols [dense 0:128 | cg 128:256] ----
                for j, b in enumerate(bs):
                    vc = vcp.tile([128, NCH, 128], BF16, tag="vc")
                    nc.gpsimd.tensor_tensor(
                        out=vc[:],
                        in0=kvb[:, j, :, 1, :],
                        in1=c_t[:, :, ds(j, 1)].broadcast_to([128, NCH, 128]),
                        op=ALU.mult,
                    )
                    vccg = smallp.tile([8, 128], BF16, tag="vccg")
                    nc.scalar.activation(
                        out=vccg[:],
                        in_=cgkv4[:, b, 128:256],
                        func=ACTF.Copy,
                        scale=ccg_all[:, ds(b, 1)],
                    )

                    jp = psJ.tile([128, 512], FP32, tag="jp")
                    for c in range(NCH):
                        nc.tensor.matmul(
                            jp[:, 0:128],
                            lhsT=vc[:, c, :],
                            rhs=kvb[:, j, c, 0, :],
                            start=(c == 0),
                            stop=False,
                            skip_group_check=True,
                        )
                    nc.tensor.matmul(
                        jp[:, 0:128],
                        lhsT=okb2[:, ds(256 * j + 128, 128)],
                        rhs=okb2[:, ds(256 * j, 128)],
                        start=False,
                        stop=True,
                        skip_group_check=True,
                    )
                    nc.tensor.matmul(
                        jp[:, 128:256],
                        lhsT=vccg,
                        rhs=cgkv2[:, b, 0:128],
                        start=True,
                        stop=False,
                        skip_group_check=True,
                    )
                    nc.tensor.matmul(
                        jp[:, 128:256],
                        lhsT=okcg2[:, ds(256 * j + 128, 128)],
                        rhs=okcg2[:, ds(256 * j, 128)],
                        start=False,
                        stop=True,
                        skip_group_check=True,
                    )

                    jsb = jsbp.tile([128, 256], FP32, tag="jsb")
                    nc.scalar.copy(jsb[:], jp[:, 0:256])

                    s1 = scrp.tile([128, 128], FP32, tag="s1")
                    nc.vector.scalar_tensor_tensor(
                        out=s1, in0=jsb[:, 0:128], scalar=1.0,
                        in1=jsb[:, 128:256],
                        op0=ALU.mult, op1=ALU.mult,
                        accum_out=accum[:, ds(b, 1)],
                    )
                    s3 = scrp.tile([128, 128], FP32, tag="s3")
                    nc.vector.scalar_tensor_tensor(
                        out=s3, in0=jsb[:, 0:128], scalar=1.0,
                        in1=jsb[:, 0:128],
                        op0=ALU.mult, op1=ALU.mult,
                        accum_out=accum[:, ds(32 + b, 1)],
                    )
                    s2 = scrp.tile([128, 128], FP32, tag="s2")
                    nc.vector.scalar_tensor_tensor(
                        out=s2, in0=jsb[:, 128:256], scalar=1.0,
                        in1=jsb[:, 128:256],
                        op0=ALU.mult, op1=ALU.mult,
                        accum_out=accum[:, ds(64 + b, 1)],
                    )

            # ---------- final reduction ----------
            # partition reduction via ones-vector matmul (gpsimd C-reduce is
            # pathologically slow on hardware)
            rps = psJ.tile([1, 128], FP32, tag="jp")
            nc.tensor.matmul(
                rps, lhsT=ones1, rhs=accum, start=True, stop=True,
                skip_group_check=True,
            )
            row = constp.tile([1, 128], FP32)
            nc.scalar.copy(row, rps)
            f1 = constp.tile([1, 32], FP32)
            nc.vector.tensor_tensor(
                out=f1, in0=row[:, 32:64], in1=row[:, 64:96], op=ALU.mult
            )
            nc.scalar.activation(out=f1, in_=f1, func=ACTF.Sqrt)
            nc.vector.tensor_scalar_add(f1, f1, EPS)
            f2 = constp.tile([1, 32], FP32)
            nc.vector.reciprocal(f2, f1)
            nc.vector.tensor_tensor(
                out=f2, in0=row[:, 0:32], in1=f2, op=ALU.mult
            )
            csum = constp.tile([1, 1], FP32)
            nc.vector.tensor_reduce(out=csum, in_=f2, axis=AX.X, op=ALU.add)
            msum = constp.tile([1, 1], FP32)
            nc.vector.tensor_reduce(
                out=msum, in_=row[:, 96:128], axis=AX.X, op=ALU.add
            )
            part = constp.tile([1, 2], FP32)
            nc.vector.tensor_scalar(
                out=part[:, 0:1], in0=csum, scalar1=-1.0, scalar2=float(B),
                op0=ALU.mult, op1=ALU.add,
            )
            nc.vector.tensor_scalar_mul(part[:, 1:2], msum, 1.0 / QD)
            nc.sync.dma_start(out=out_d, in_=part)

    return nc


_NC_CACHE = {}


def _get_nc():
    if "nc" not in _NC_CACHE:
        _install_drain_fix()
        nc = build_nc()
        _split_waits(nc)
        _NC_CACHE["nc"] = nc
    return _NC_CACHE["nc"]


def _split_waits(nc):
    """This walrus accepts only one sync-wait per instruction; move extras
    onto same-engine NoOps inserted just before."""
    from concourse import mybir
    from bass_rust import SyncInfo

    for f in nc.m.functions:
        for blk in f.blocks:
            insts = list(blk.instructions)
            out = []
            for inst in insts:
                si = inst.sync_info
                waits = list(si.on_wait) if si is not None else []
                if len(waits) > 1:
                    for wi, w in enumerate(waits[:-1]):
                        nop = mybir.InstNoOp(name=f"{inst.name}-wsplit{wi}")
                        nop.engine = inst.engine
                        nop.sync_info = SyncInfo(on_wait=[w], on_update=[])
                        out.append(nop)
                    inst.sync_info = SyncInfo(
                        on_wait=[waits[-1]], on_update=list(si.on_update)
                    )
                out.append(inst)
            blk.instructions = out


N_CORES = 8


def kernel(queries, keys, values, k_cg, v_cg):
    from concourse.bass_utils import run_bass_kernel_spmd

    queries = np.ascontiguousarray(np.asarray(queries, dtype=np.float32))
    keys = np.ascontiguousarray(np.asarray(keys, dtype=np.float32))
    values = np.ascontiguousarray(np.asarray(values, dtype=np.float32))
    k_cg = np.ascontiguousarray(np.asarray(k_cg, dtype=np.float32))
    v_cg = np.ascontiguousarray(np.asarray(v_cg, dtype=np.float32))

    nb = queries.shape[0]
    sh = nb // N_CORES
    in_maps = [
        {
            "queries": queries[i * sh : (i + 1) * sh],
            "keys": keys[i * sh : (i + 1) * sh],
            "values": values[i * sh : (i + 1) * sh],
            "k_cg": k_cg[i * sh : (i + 1) * sh],
            "v_cg": v_cg[i * sh : (i + 1) * sh],
        }
        for i in range(N_CORES)
    ]
    nc = _get_nc()
    res = run_bass_kernel_spmd(nc, in_maps, core_ids=list(range(N_CORES)))
    total = 0.0
    for i in range(N_CORES):
        part = res.results[i]["out"]
        total += float(part[0, 0]) + float(part[0, 1])
    return np.float32(total / nb)


# revision 18
# speedup vs baseline: 1.2801x; 1.0876x over previous
"""Trainium2 Bass kernel for nn_ForceMatchingLoss (batch-data-parallel over 8 NeuronCores).

Full inputs (B=256) are sharded along the batch dimension: core i computes
batches [32*i, 32*i+32) and returns [sum_b fd_b, sum_b cons_b]; the host
sums the 8 partials and divides by 256 (the loss is a batch mean, so the
"all-reduce" is a trivial host-side sum of 8 scalars).

v2: full-bf16 matmul pipeline (fp32 inputs are cast to bf16 on-chip).  The
fp32 LOW_HIGH 2-pass matmuls/transposes of v1 dominated the tensor engine
(~166us busy); bf16 runs single-pass with fast weight loads.  PSUM
accumulation stays fp32 and the final reductions stay fp32, keeping the
loss within ~1e-5 of the fp64 reference.
"""

import numpy as np


# ---------------------------------------------------------------------------
# Workaround for this walrus build: CTRL-type instructions (Drain) only accept
# a single sync-wait; TileContext's tail drain aggregates one wait per logical
# processor.  Split the waits across a chain of drains.
# ---------------------------------------------------------------------------
def _install_drain_fix():
    import concourse.tile as tile
    from bass_rust import ScopedClock, SyncInfo

    if getattr(tile.TileContext, "_drain_fix_installed", False):
        return

    def _drain_and_barrier(self, tick_clock, wait_clock):
        drain_inst = self.nc.sync.drain()
        wait_clock.add_sem_waits(
            drain_inst.ins, ScopedClock({None: tick_clock.global_clock})
        )
        si = drain_inst.ins.sync_info
        waits = list(si.on_wait) if si is not None else []
        if len(waits) > 1:
            drain_inst.ins.sync_info = SyncInfo(
                on_wait=waits[:1], on_update=list(si.on_update)
            )
            for i in range(1, len(waits)):
                d = self.nc.sync.drain()
                d.ins.sync_info = SyncInfo(on_wait=waits[i : i + 1], on_update=[])

        self.nc.all_engine_barrier()
        popped = self.nc._tile_sem_poison_stack.pop()
        assert popped is self._sem_poison
        self.nc.clear_and_free_semaphores(list(self.sems.allocated().values()))
        self.nc.all_engine_barrier()

    tile.TileContext._drain_and_barrier = _drain_and_barrier
    tile.TileContext._drain_fix_installed = True


import concourse.bass as bass
import concourse.tile as tile
from concourse import mybir
from concourse.bass import ds, ts
from concourse.masks import make_identity

FP32 = mybir.dt.float32
BF16 = mybir.dt.bfloat16
AX = mybir.AxisListType
ALU = mybir.AluOpType
ACTF = mybir.ActivationFunctionType

B = 32          # batches per core
Q = 16
S = 512
M = 8
D = 128
NCH = 4         # s chunks of 128
GB = 4          # batches per group (packed at 32-row offsets in psum)
NG = B // GB    # 8 groups
SCALE = float(D) ** -0.5
EPS = 1e-8
QD = float(Q * D)


def build_nc():
    nc = bass.Bass("TRN2", target_bir_lowering=False, debug=False)
    q_d = nc.dram_tensor("queries", [B, Q, D], FP32, kind="ExternalInput").ap()
    k_d = nc.dram_tensor("keys", [B, S, D], FP32, kind="ExternalInput").ap()
    v_d = nc.dram_tensor("values", [B, S, D], FP32, kind="ExternalInput").ap()
    kcg_d = nc.dram_tensor("k_cg", [B, M, D], FP32, kind="ExternalInput").ap()
    vcg_d = nc.dram_tensor("v_cg", [B, M, D], FP32, kind="ExternalInput").ap()
    out_d = nc.dram_tensor("out", [1, 2], FP32, kind="ExternalOutput").ap()

    with tile.TileContext(nc) as tc:
        with (
            tc.tile_pool(name="const", bufs=1) as constp,
            tc.tile_pool(name="kv4", bufs=3) as kv4p,
            tc.tile_pool(name="kvb", bufs=3) as kvbp,
            tc.tile_pool(name="kt", bufs=6) as ktp,
            tc.tile_pool(name="sm", bufs=4) as smp,
            tc.tile_pool(name="small", bufs=4) as smallp,
            tc.tile_pool(name="vc", bufs=3) as vcp,
            tc.tile_pool(name="jsb", bufs=4) as jsbp,
            tc.tile_pool(name="okb", bufs=3) as okbp,
            tc.tile_pool(name="scr", bufs=6) as scrp,
            tc.tile_pool(name="psA", bufs=2, space="PSUM") as psA,
            tc.tile_pool(name="psS", bufs=1, space="PSUM") as psS,
            tc.tile_pool(name="psOK", bufs=2, space="PSUM") as psOK,
            tc.tile_pool(name="psJ", bufs=2, space="PSUM") as psJ,
            tc.tile_pool(name="psCG", bufs=1, space="PSUM") as psCG,
        ):
            ident = constp.tile([128, 128], FP32)
            make_identity(nc, ident)
            identB = constp.tile([128, 128], BF16)
            nc.scalar.copy(identB, ident)
            zeroTb = constp.tile([128, 128], BF16)
            nc.scalar.activation(out=zeroTb, in_=ident, func=ACTF.Copy, scale=0.0)
            # accumulator columns: [dot 0:32 | d2 32:64 | c2 64:96 | cons 96:128]
            accum = constp.tile([128, 128], FP32)
            nc.gpsimd.memset(accum, 0.0)
            ones1 = constp.tile([128, 1], FP32)
            nc.vector.memset(ones1, 1.0)
            ccg_all = constp.tile([8, 32], FP32)

            # ---------- prologue: q transposed, cg tensors ----------
            q_sb = constp.tile([128, 4, 128], FP32)
            nc.sync.dma_start(
                out=q_sb,
                in_=q_d.rearrange("(t b2) q d -> (b2 q) t d", t=4),
            )
            q_sbb = constp.tile([128, 4, 128], BF16)
            nc.scalar.copy(q_sbb, q_sb)
            qT = constp.tile([128, 4, 128], BF16)  # [d, t, b2*16+q], scaled
            qtps = psA.tile([128, 1024], BF16, tag="tp")
            for t in range(4):
                nc.tensor.transpose(qtps[:, ds(128 * t, 128)], q_sbb[:, t, :], identB)
            # fold the softmax temperature into qT (used by dense + cg scores)
            nc.scalar.activation(
                out=qT[:],
                in_=qtps[:, 0:512].rearrange("p (t x) -> p t x", t=4),
                func=ACTF.Copy,
                scale=SCALE,
            )

            kcg_sb = constp.tile([128, 2, 128], FP32)  # [(b2 m), t, d]
            nc.sync.dma_start(
                out=kcg_sb,
                in_=kcg_d.rearrange("(t b2) m d -> (b2 m) t d", t=2),
            )
            kcg_sbb = constp.tile([128, 2, 128], BF16)
            nc.scalar.copy(kcg_sbb, kcg_sb)
            kcgT = constp.tile([128, 2, 128], BF16)  # [d, t, b2*8+m]
            kcgtps = psA.tile([128, 1024], BF16, tag="tp")
            for t in range(2):
                nc.tensor.transpose(
                    kcgtps[:, ds(128 * t, 128)], kcg_sbb[:, t, :], identB
                )
            nc.scalar.copy(
                kcgT[:], kcgtps[:, 0:256].rearrange("p (t x) -> p t x", t=2)
            )

            # [m, b, {k|v}]: fp32 HWDGE load, then one SWDGE SBUF->SBUF
            # cast to bf16 (large contiguous descriptors, cheap on Q7)
            cgkv4 = constp.tile([8, 32, 256], FP32)
            nc.sync.dma_start(
                out=cgkv4[:, :, 0:128], in_=kcg_d.rearrange("b m d -> m b d")
            )
            nc.sync.dma_start(
                out=cgkv4[:, :, 128:256], in_=vcg_d.rearrange("b m d -> m b d")
            )
            cgkv2 = constp.tile([8, 32, 256], BF16)
            nc.gpsimd.dma_start(out=cgkv2, in_=cgkv4)

            # ---------- main loop over groups of 4 batches ----------
            def load_group(g):
                # fp32 staging loads (per-batch 256KB DMAs; the AP balancer
                # cannot handle the 4-dim whole-group transfer), then cast to
                # bf16 (vector CAST is by far the cheapest; gpsimd CAST
                # measured 3.7us -- never use it)
                kv4 = kv4p.tile([128, GB, NCH, 2, 128], FP32, tag="kv4")
                for j in range(GB):
                    b = g * GB + j
                    nc.sync.dma_start(
                        out=kv4[:, j, :, 0, :],
                        in_=k_d[b].rearrange("(p c) d -> p c d", c=NCH),
                    )
                    nc.sync.dma_start(
                        out=kv4[:, j, :, 1, :],
                        in_=v_d[b].rearrange("(p c) d -> p c d", c=NCH),
                    )
                kvb = kvbp.tile([128, GB, NCH, 2, 128], BF16, tag="kvb")
                nc.scalar.copy(kvb[:, 0], kv4[:, 0])
                nc.vector.tensor_copy(kvb[:, 1], kv4[:, 1])
                nc.vector.tensor_copy(kvb[:, 2], kv4[:, 2])
                nc.vector.tensor_copy(kvb[:, 3], kv4[:, 3])
                return kvb

            kvb_next = load_group(0)
            for g in range(NG):
                bs = [g * GB + j for j in range(GB)]
                kvb = kvb_next
                if g + 1 < NG:
                    kvb_next = load_group(g + 1)

                # kT per batch via PE transpose (bf16 single-pass)
                kts = []
                for j in range(GB):
                    ktps = psA.tile([128, 1024], BF16, tag="tp")
                    for c in range(NCH):
                        nc.tensor.transpose(
                            ktps[:, ds(128 * c, 128)], kvb[:, j, c, 0, :], identB
                        )
                    kt = ktp.tile([128, NCH, 128], BF16, tag="kt")
                    view = ktps[:, 0:512].rearrange("p (c x) -> p c x", c=NCH)
                    if j % 2 == 0:
                        nc.scalar.copy(kt[:], view)
                    else:
                        nc.vector.tensor_copy(kt[:], view)
                    kts.append(kt)

                # scores: 4 batches packed at 32-aligned row offsets
                scps = psS.tile([128, 512], FP32, tag="scps")
                nc.tensor.matmul(
                    scps,
                    lhsT=zeroTb,
                    rhs=kts[0][:],
                    start=True,
                    stop=False,
                    skip_group_check=True,
                )
                for j, b in enumerate(bs):
                    t, i = b // 8, b % 8
                    nc.tensor.matmul(
                        scps[ds(32 * j, 16), :],
                        lhsT=qT[:, t, ds(16 * i, 16)],
                        rhs=kts[j][:],
                        start=False,
                        stop=True,
                        tile_position=(0, 32 * j),
                        skip_group_check=True,
                    )

                # softmax (no max subtraction: |scores| <= ~7)
                ptil = smp.tile([128, 512], BF16, tag="ptil")
                z = smallp.tile([128, 1], FP32, tag="z")
                nc.scalar.activation(out=ptil, in_=scps, func=ACTF.Exp, accum_out=z)
                zr = smallp.tile([128, 1], FP32, tag="zr")
                nc.vector.reciprocal(zr, z)
                nc.vector.tensor_scalar_mul(ptil, ptil, zr)

                # pT via PE transpose
                ptps = psA.tile([128, 1024], BF16, tag="tp")
                for c in range(NCH):
                    nc.tensor.transpose(
                        ptps[:, ds(128 * c, 128)], ptil[:, ds(128 * c, 128)], identB
                    )
                pT = smp.tile([128, NCH, 128], BF16, tag="pT")
                nc.scalar.copy(
                    pT[:], ptps[:, 0:512].rearrange("p (c x) -> p c x", c=NCH)
                )

                # c = sum_q p (valid q columns only).  NOTE: the jacobians are
                # computed WITHOUT the overall SCALE factor -- cosine
                # similarity is invariant to a uniform scale on each jacobian,
                # so SCALE is dropped from c/vc/ccg/vccg and the jac2 kbar
                # copies use -1.0 below.
                c_t = smallp.tile([128, NCH, GB], FP32, tag="c_t")
                nc.vector.tensor_reduce(
                    out=c_t,
                    in_=pT.rearrange("p c (j w) -> p c j w", j=GB)[:, :, :, 0:16],
                    axis=AX.X,
                    op=ALU.add,
                )

                # out/kbar: 4 batches col-packed at 32-row offsets in one bank;
                # cols [kbar 0:128 | out 128:256]
                okps = psOK.tile([128, 512], FP32, tag="okps")
                nc.tensor.matmul(
                    okps[:, 0:256],
                    lhsT=zeroTb,
                    rhs=kvb[:, 0, 0].rearrange("p a x -> p (a x)"),
                    start=True,
                    stop=False,
                    skip_group_check=True,
                )
                for c in range(NCH):
                    for j in range(GB):
                        nc.tensor.matmul(
                            okps[ds(32 * j, 16), 0:256],
                            lhsT=pT[:, c, ds(32 * j, 16)],
                            rhs=kvb[:, j, c].rearrange("p a x -> p (a x)"),
                            start=False,
                            stop=(c == NCH - 1),
                            tile_position=(0, 32 * j),
                            skip_group_check=True,
                        )
                # okb rows 32j: [-s*kbar 0:128 | out 128:256] (bf16)
                okb = okbp.tile([128, 256], BF16, tag="okb")
                nc.scalar.activation(
                    out=okb[:, 0:128], in_=okps[:, 0:128], func=ACTF.Copy,
                    scale=-1.0,
                )
                nc.scalar.copy(okb[:, 128:256], okps[:, 128:256])
                # move each batch's 16 rows down to partitions 0:16 so the
                # consistency sub and jac2 see base-partition-0 operands
                okb2 = okbp.tile([16, 1024], BF16, tag="okb2")
                for j in range(GB):
                    nc.sync.dma_start(
                        out=okb2[:, ds(256 * j, 256)], in_=okb[ds(32 * j, 16), :]
                    )

                # ---- coarse-grained chain ----
                cgps = psCG.tile([128, 512], FP32, tag="cgps")
                nc.tensor.matmul(
                    cgps[:, 0:8],
                    lhsT=zeroTb,
                    rhs=kcgT[:, 0, 0:8],
                    start=True,
                    stop=False,
                    skip_group_check=True,
                )
                for j, b in enumerate(bs):
                    t2, i2 = b // 16, b % 16
                    nc.tensor.matmul(
                        cgps[ds(32 * j, 16), 0:8],
                        lhsT=qT[:, b // 8, ds(16 * (b % 8), 16)],
                        rhs=kcgT[:, t2, ds(8 * i2, 8)],
                        start=False,
                        stop=True,
                        tile_position=(0, 32 * j),
                        skip_group_check=True,
                    )
                pcgf = smallp.tile([128, 8], FP32, tag="pcgf")
                zcg = smallp.tile([128, 1], FP32, tag="zcg")
                nc.scalar.activation(
                    out=pcgf, in_=cgps[:, 0:8], func=ACTF.Exp, accum_out=zcg
                )
                zcgr = smallp.tile([128, 1], FP32, tag="zcgr")
                nc.vector.reciprocal(zcgr, zcg)
                pcg = smallp.tile([128, 8], BF16, tag="pcg")
                nc.vector.tensor_scalar_mul(pcg, pcgf, zcgr)
                pcgtps = psA.tile([128, 1024], BF16, tag="tp")
                nc.tensor.transpose(pcgtps[0:8, 0:128], pcg, identB)
                pcgT = smallp.tile([8, 128], BF16, tag="pcgT")
                nc.scalar.copy(pcgT[:], pcgtps[0:8, 0:128])
                nc.vector.tensor_reduce(
                    out=ccg_all[:, ds(GB * g, GB)],
                    in_=pcgT.rearrange("m (j w) -> m j w", j=GB)[:, :, 0:16],
                    axis=AX.X,
                    op=ALU.add,
                )
                # cg out/kbar: psum cols 256:512 = [kbar_cg | out_cg], rows 32j
                nc.tensor.matmul(
                    cgps[:, 256:512],
                    lhsT=zeroTb[0:8, :],
                    rhs=cgkv2[:, bs[0], :],
                    start=True,
                    stop=False,
                    skip_group_check=True,
                )
                for j, b in enumerate(bs):
                    nc.tensor.matmul(
                        cgps[ds(32 * j, 16), 256:512],
                        lhsT=pcgT[:, ds(32 * j, 16)],
                        rhs=cgkv2[:, b, :],
                        start=False,
                        stop=True,
                        tile_position=(0, 32 * j),
                        skip_group_check=True,
                    )
                # okcg rows 32j: [-s*kbar_cg 0:128 | out_cg 128:256] (bf16)
                okcg = okbp.tile([128, 256], BF16, tag="okcg")
                nc.scalar.activation(
                    out=okcg[:, 0:128], in_=cgps[:, 256:384], func=ACTF.Copy,
                    scale=-1.0,
                )
                nc.scalar.copy(okcg[:, 128:256], cgps[:, 384:512])
                okcg2 = okbp.tile([16, 1024], BF16, tag="okcg2")
                for j in range(GB):
                    nc.sync.dma_start(
                        out=okcg2[:, ds(256 * j, 256)], in_=okcg[ds(32 * j, 16), :]
                    )

                # consistency per batch: out (okb2) vs out_cg (okcg2), rows 0:16
                for j, b in enumerate(bs):
                    dif = scrp.tile([16, 128], BF16, tag="dif")
                    nc.vector.tensor_sub(
                        dif,
                        okb2[:, ds(256 * j + 128, 128)],
                        okcg2[:, ds(256 * j + 128, 128)],
                    )
                    scc = scrp.tile([16, 128], BF16, tag="scc")
                    nc.vector.scalar_tensor_tensor(
                        out=scc,
                        in0=dif,
                        scalar=1.0,
                        in1=dif,
                        op0=ALU.mult,
                        op1=ALU.mult,
                        accum_out=accum[0:16, ds(96 + b, 1)],
                    )

                # ---- per-batch jacobians: jp cols [dense 0:128 | cg 128:256] ----
                for j, b in enumerate(bs):
                    vc = vcp.tile([128, NCH, 128], BF16, tag="vc")
                    nc.gpsimd.tensor_tensor(
                        out=vc[:],
                        in0=kvb[:, j, :, 1, :],
                        in1=c_t[:, :, ds(j, 1)].broadcast_to([128, NCH, 128]),
                        op=ALU.mult,
                    )
                    vccg = smallp.tile([8, 128], BF16, tag="vccg")
                    nc.scalar.activation(
                        out=vccg[:],
                        in_=cgkv4[:, b, 128:256],
                        func=ACTF.Copy,
                        scale=ccg_all[:, ds(b, 1)],
                    )

                    jp = psJ.tile([128, 512], FP32, tag="jp")
                    for c in range(NCH):
                        nc.tensor.matmul(
                            jp[:, 0:128],
                            lhsT=vc[:, c, :],
                            rhs=kvb[:, j, c, 0, :],
                            start=(c == 0),
                            stop=False,
                            skip_group_check=True,
                        )
                    nc.tensor.matmul(
                        jp[:, 0:128],
                        lhsT=okb2[:, ds(256 * j + 128, 128)],
                        rhs=okb2[:, ds(256 * j, 128)],
                        start=False,
                        stop=True,
                        skip_group_check=True,
                    )
                    nc.tensor.matmul(
                        jp[:, 128:256],
                        lhsT=vccg,
                        rhs=cgkv2[:, b, 0:128],
                        start=True,
                        stop=False,
                        skip_group_check=True,
                    )
                    nc.tensor.matmul(
                        jp[:, 128:256],
                        lhsT=okcg2[:, ds(256 * j + 128, 128)],
                        rhs=okcg2[:, ds(256 * j, 128)],
                        start=False,
                        stop=True,
                        skip_group_check=True,
                    )

                    jsb = jsbp.tile([128, 256], FP32, tag="jsb")
                    nc.scalar.copy(jsb[:], jp[:, 0:256])

                    s1 = scrp.tile([128, 128], BF16, tag="s1")
                    nc.vector.scalar_tensor_tensor(
                        out=s1, in0=jsb[:, 0:128], scalar=1.0,
                        in1=jsb[:, 128:256],
                        op0=ALU.mult, op1=ALU.mult,
                        accum_out=accum[:, ds(b, 1)],
                    )
                    s3 = scrp.tile([128, 128], BF16, tag="s3")
                    nc.vector.scalar_tensor_tensor(
                        out=s3, in0=jsb[:, 0:128], scalar=1.0,
                        in1=jsb[:, 0:128],
                        op0=ALU.mult, op1=ALU.mult,
                        accum_out=accum[:, ds(32 + b, 1)],
                    )
                    s2 = scrp.tile([128, 128], BF16, tag="s2")
                    nc.vector.scalar_tensor_tensor(
                        out=s2, in0=jsb[:, 128:256], scalar=1.0,
                        in1=jsb[:, 128:256],
                        op0=ALU.mult, op1=ALU.mult,
                        accum_out=accum[:, ds(64 + b, 1)],
                    )

            # ---------- final reduction ----------
            # partition reduction via ones-vector matmul (gpsimd C-reduce is
            # pathologically slow on hardware)
            rps = psJ.tile([1, 128], FP32, tag="jp")
            nc.tensor.matmul(
                rps, lhsT=ones1, rhs=accum, start=True, stop=True,
                skip_group_check=True,
            )
            row = constp.tile([1, 128], FP32)
            nc.scalar.copy(row, rps)
            f1 = constp.tile([1, 32], FP32)
            nc.vector.tensor_tensor(
                out=f1, in0=row[:, 32:64], in1=row[:, 64:96], op=ALU.mult
            )
            nc.scalar.activation(out=f1, in_=f1, func=ACTF.Sqrt)
            nc.vector.tensor_scalar_add(f1, f1, EPS)
            f2 = constp.tile([1, 32], FP32)
            nc.vector.reciprocal(f2, f1)
            nc.vector.tensor_tensor(
                out=f2, in0=row[:, 0:32], in1=f2, op=ALU.mult
            )
            csum = constp.tile([1, 1], FP32)
            nc.vector.tensor_reduce(out=csum, in_=f2, axis=AX.X, op=ALU.add)
            msum = constp.tile([1, 1], FP32)
            nc.vector.tensor_reduce(
                out=msum, in_=row[:, 96:128], axis=AX.X, op=ALU.add
            )
            part = constp.tile([1, 2], FP32)
            nc.vector.tensor_scalar(
                out=part[:, 0:1], in0=csum, scalar1=-1.0, scalar2=float(B),
                op0=ALU.mult, op1=ALU.add,
            )
            nc.vector.tensor_scalar_mul(part[:, 1:2], msum, 1.0 / QD)
            nc.sync.dma_start(out=out_d, in_=part)

    return nc


_NC_CACHE = {}


def _get_nc():
    if "nc" not in _NC_CACHE:
        _install_drain_fix()
        nc = build_nc()
        _split_waits(nc)
        _NC_CACHE["nc"] = nc
    return _NC_CACHE["nc"]


def _split_waits(nc):
    """This walrus accepts only one sync-wait per instruction; move extras
    onto same-engine NoOps inserted just before."""
    from concourse import mybir
    from bass_rust import SyncInfo

    for f in nc.m.functions:
        for blk in f.blocks:
            insts = list(blk.instructions)
            out = []
            for inst in insts:
                si = inst.sync_info
                waits = list(si.on_wait) if si is not None else []
                if len(waits) > 1:
                    for wi, w in enumerate(waits[:-1]):
                        nop = mybir.InstNoOp(name=f"{inst.name}-wsplit{wi}")
                        nop.engine = inst.engine
                        nop.sync_info = SyncInfo(on_wait=[w], on_update=[])
                        out.append(nop)
                    inst.sync_info = SyncInfo(
                        on_wait=[waits[-1]], on_update=list(si.on_update)
                    )
                out.append(inst)
            blk.instructions = out


N_CORES = 8


def kernel(queries, keys, values, k_cg, v_cg):
    from concourse.bass_utils import run_bass_kernel_spmd

    queries = np.ascontiguousarray(np.asarray(queries, dtype=np.float32))
    keys = np.ascontiguousarray(np.asarray(keys, dtype=np.float32))
    values = np.ascontiguousarray(np.asarray(values, dtype=np.float32))
    k_cg = np.ascontiguousarray(np.asarray(k_cg, dtype=np.float32))
    v_cg = np.ascontiguousarray(np.asarray(v_cg, dtype=np.float32))

    nb = queries.shape[0]
    sh = nb // N_CORES
    in_maps = [
        {
            "queries": queries[i * sh : (i + 1) * sh],
            "keys": keys[i * sh : (i + 1) * sh],
            "values": values[i * sh : (i + 1) * sh],
            "k_cg": k_cg[i * sh : (i + 1) * sh],
            "v_cg": v_cg[i * sh : (i + 1) * sh],
        }
        for i in range(N_CORES)
    ]
    nc = _get_nc()
    res = run_bass_kernel_spmd(nc, in_maps, core_ids=list(range(N_CORES)))
    total = 0.0
    for i in range(N_CORES):
        part = res.results[i]["out"]
        total += float(part[0, 0]) + float(part[0, 1])
    return np.float32(total / nb)
